# revision 48
# baseline (speedup 1.0000x reference)
"""Trainium2 8-core tensor-parallel attention kernel (Bass/Tile).

Strategy (TP over heads, per the ColumnParallel/RowParallel intent):
  - Each of the 8 cores owns 1 KV head and its 4 GQA query heads.
  - Phase A1: Q/K/V projections for all 8 512-token slots (bf16 matmuls,
    fp32 PSUM), RoPE on-chip; Q^T/K^T/V kept SBUF-resident.
  - Phase A2: causal attention, head-outer order, no max-subtraction
    (scores are bounded so exp is safe in fp32); O^T = V^T P^T accumulated
    in PSUM over key blocks; per-token softmax denominators via an
    all-ones stationary matmul (replicated across partitions). The key-
    block loop is software-pipelined one deep so the ACT-engine exp never
    stalls the TensorEngine. After each head finishes, a per-head
    AllToAll fires, overlapping later heads' attention.
  - Phase C: the output projection is sequence-parallel: the AllToAlls
    convert head-sharding to token-sharding, then each core multiplies
    its 512-token slice by the FULL wo. No all-reduce; the host
    concatenates the 8 disjoint token slices.

Layout choices:
  - Activations stay transposed (X^T/Q^T/K^T/O^T: features on the 128
    partitions, tokens on the free axis) so every matmul streams 512 wide
    and fp32 DMA-transposes are never needed; V is transposed to natural
    token-major via TensorE transpose (cheap).
  - RoPE pairs are de-interleaved on the host by permuting wq/wk columns
    (even lanes then odd lanes within each head) -> RoPE is 6 vector ops
    on partition halves. The permutation cancels in Q.K^T.
  - All matmul operands are bf16 (fp32 PSUM accumulation).
"""

import math

import numpy as np

B, S, D = 2, 2048, 4096
NH, NKV, HD = 32, 8, 128
N_REP = NH // NKV
N_CORES = 8
TOK = B * S            # 4096 flattened tokens
TQ = 512               # query-block width (matmul moving free dim)
TKB = 128              # key-block width (stationary free dim)
NKC = D // 128         # 32 contraction chunks of 128
NQB = S // TQ          # 4 query blocks per batch
NSLOT = B * NQB        # 8 512-token slots
NH_LOC = NH // N_CORES  # 4 query heads per core
SCALE = 1.0 / math.sqrt(HD)

PROFILE = False
TRACE_DIR = None
LAST_EXEC_NS = None
LAST_TRACE_DIR = None

_NC_CACHE = {}


def _build_nc(mode: str):
    """mode: 'causal' (skip fully-masked blocks, triangular diag masks),
    'dense' (no mask at all), 'masked' (generic additive mask from DRAM)."""
    import concourse.tile as tile
    from concourse import bacc, mybir
    from concourse.masks import make_identity

    f32 = mybir.dt.float32
    bf16 = mybir.dt.bfloat16

    nc = bacc.Bacc(None, target_bir_lowering=False, num_devices=N_CORES)

    xT = nc.declare_dram_parameter("xT", [NSLOT, 128, NKC, TQ], bf16, isOutput=False)
    wq = nc.declare_dram_parameter("wq", [128, NKC, NH_LOC * HD], bf16, isOutput=False)
    wk = nc.declare_dram_parameter("wk", [128, NKC, HD], bf16, isOutput=False)
    wv = nc.declare_dram_parameter("wv", [128, NKC, HD], bf16, isOutput=False)
    # wo rows split (s_, hg): row k = 4*s_ + hg of the original [nb, 128, NH, TQ]
    wo = nc.declare_dram_parameter("wo", [D // TQ, 128, N_CORES, NH_LOC, TQ], bf16,
                                   isOutput=False)
    cosT = nc.declare_dram_parameter("cosT", [64, TOK], f32, isOutput=False)
    sinT = nc.declare_dram_parameter("sinT", [64, TOK], f32, isOutput=False)
    if mode == "causal":
        masktri = nc.declare_dram_parameter("masktri", [128, 128], bf16,
                                            isOutput=False)
    if mode == "masked":
        maskT = nc.declare_dram_parameter("maskT", [S, S], f32, isOutput=False)
    out = nc.declare_dram_parameter("out", [TQ, D], f32, isOutput=True)

    with tile.TileContext(nc) as tc:
        from contextlib import ExitStack

        with (
            tc.tile_pool(name="dram", bufs=1, space="DRAM") as dram,
        ):
            a2a_in = [
                dram.tile([N_CORES, 128, TQ], bf16, name=f"a2a_in{h}")
                for h in range(NH_LOC)
            ]
            a2a_out = [
                dram.tile([N_CORES, 128, TQ], bf16, name=f"a2a_out{h}")
                for h in range(NH_LOC)
            ]

            actx = ExitStack()
            singles = actx.enter_context(tc.tile_pool(name="singles", bufs=1))
            kvp = actx.enter_context(tc.tile_pool(name="kvp", bufs=1))
            work = actx.enter_context(tc.tile_pool(name="work", bufs=3))
            psctx = ExitStack()
            pp = psctx.enter_context(tc.tile_pool(name="pp", bufs=2, space="PSUM"))
            pacc = psctx.enter_context(
                tc.tile_pool(name="pacc", bufs=2, space="PSUM")
            )
            psums = psctx.enter_context(
                tc.tile_pool(name="psums", bufs=2, space="PSUM")
            )
            xtctx = ExitStack()
            xtp = xtctx.enter_context(tc.tile_pool(name="xtp", bufs=2))

            # ---- resident weights/constants, load order = first-use order:
            # wk + slot-0 activations first so the PE starts ~immediately,
            # then wv (needed ~7us in), wq (needed ~14us in), then the rest.
            def load_xt(j):
                # host pre-tiled per slot: per-partition contiguous 32KB
                xt_t = xtp.tile([128, NKC, TQ], bf16, tag="xt", name=f"xt{j}")
                # halves on both rings so each slot's data lands in ~half
                # the single-ring time; cos/sin ride the idle gpsimd ring
                nc.sync.dma_start(xt_t[:, 0:16, :], xT[j, :, 0:16, :])
                nc.scalar.dma_start(xt_t[:, 16:32, :], xT[j, :, 16:32, :])
                cos_sl = work.tile([64, TQ], f32, tag="cos", bufs=2, name=f"cos{j}")
                nc.gpsimd.dma_start(cos_sl[:], cosT[:, j * TQ : (j + 1) * TQ])
                sin_sl = work.tile([64, TQ], f32, tag="sin", bufs=2, name=f"sin{j}")
                nc.gpsimd.dma_start(sin_sl[:], sinT[:, j * TQ : (j + 1) * TQ])
                return xt_t, cos_sl, sin_sl

            # startup rings (slot 0 is emitted chunk-outer, consuming each
            # arriving quarter with 6 matmuls per c-chunk, so the whole
            # 11MB initial burst streams without PE dead time):
            #   sync:   wq quarters (biggest stream, needed from c=0)
            #   scalar: xt0 quarters
            #   gpsimd: wk/wv quarters interleaved, then cos/sin
            wk_sb = singles.tile([128, NKC, HD], bf16)
            wv_sb = singles.tile([128, NKC, HD], bf16)
            wq_sb = xtp.tile([128, NKC, NH_LOC * HD], bf16, bufs=1)
            xt0_t = xtp.tile([128, NKC, TQ], bf16, tag="xt", name="xt0")

            def q4dma(eng, dst, src, q4):
                sl = slice(q4 * 8, (q4 + 1) * 8)
                eng.dma_start(dst[:, sl, :], src[:, sl, :])

            xsrc = xT[0]
            # tiny c0-1 slices first: the opening matmuls fire ~5us earlier
            nc.sync.dma_start(wq_sb[:, 0:2, :], wq[:, 0:2, :])
            nc.scalar.dma_start(xt0_t[:, 0:2, :], xsrc[:, 0:2, :])
            nc.gpsimd.dma_start(wk_sb[:, 0:2, :], wk[:, 0:2, :])
            nc.gpsimd.dma_start(wv_sb[:, 0:2, :], wv[:, 0:2, :])
            nc.sync.dma_start(wq_sb[:, 2:8, :], wq[:, 2:8, :])
            nc.scalar.dma_start(xt0_t[:, 2:8, :], xsrc[:, 2:8, :])
            nc.gpsimd.dma_start(wk_sb[:, 2:8, :], wk[:, 2:8, :])
            nc.gpsimd.dma_start(wv_sb[:, 2:8, :], wv[:, 2:8, :])
            q4dma(nc.gpsimd, xt0_t, xsrc, 1)    # c=8
            q4dma(nc.scalar, wq_sb, wq, 1)      # c=8
            q4dma(nc.gpsimd, wk_sb, wk, 1)      # c=8
            q4dma(nc.gpsimd, wv_sb, wv, 1)      # c=8
            q4dma(nc.sync, xt0_t, xsrc, 2)      # c=16
            q4dma(nc.sync, wq_sb, wq, 2)        # c=16
            q4dma(nc.gpsimd, wk_sb, wk, 2)      # c=16
            q4dma(nc.gpsimd, wv_sb, wv, 2)      # c=16
            q4dma(nc.scalar, xt0_t, xsrc, 3)    # c=24
            q4dma(nc.scalar, wq_sb, wq, 3)      # c=24
            q4dma(nc.gpsimd, wk_sb, wk, 3)      # c=24
            q4dma(nc.gpsimd, wv_sb, wv, 3)      # c=24
            cos0 = work.tile([64, TQ], f32, tag="cos", bufs=2, name="cos0")
            nc.gpsimd.dma_start(cos0[:], cosT[:, 0:TQ])
            sin0 = work.tile([64, TQ], f32, tag="sin", bufs=2, name="sin0")
            nc.gpsimd.dma_start(sin0[:], sinT[:, 0:TQ])
            xt0 = (xt0_t, cos0, sin0)
            # all-ones stationary: ones^T @ P^T replicates the per-token key-sum
            # across all 128 PSUM partitions (avoids partition-broadcast later)
            ones_sb = singles.tile([128, 128], bf16)
            nc.vector.memset(ones_sb, 1.0)
            ident_sb = singles.tile([128, 128], bf16)
            make_identity(nc, ident_sb)
            if mode == "causal":
                # masktri[c, p] = -1e9 where p > c; masktri.T @ I adds -1e9
                # to the upper triangle of a 128x128 diag score chunk
                masktri_sb = singles.tile([128, 128], bf16)
                nc.scalar.dma_start(masktri_sb[:], masktri[:, :])

            # resident K^T [hd, tok], V natural [tk, kb, hd]; Q^T spills to
            # DRAM (tiny contiguous reloads) to keep SBUF headroom
            kres = kvp.tile([128, TOK], bf16)
            vres = kvp.tile([128, TOK // TKB, HD], bf16)
            qres = dram.tile([NH_LOC, NSLOT, 128, TQ], bf16)

            def rope(dst, ps, cos_sl, sin_sl):
                """dst[hd, t] (bf16) <- rotate(ps[hd, t]) with de-interleaved
                halves: rows 0:64 = even lanes (t0), 64:128 = odd lanes (t1).
                out0 = t0*c - t1*s ; out1 = t0*s + t1*c."""
                t0, t1 = ps[0:64], ps[64:128]
                ta = work.tile([64, TQ], f32, tag="rope_a", bufs=2)
                tb = work.tile([64, TQ], f32, tag="rope_b", bufs=2)
                nc.vector.tensor_mul(ta[:], t0, cos_sl[:])
                nc.vector.tensor_mul(tb[:], t1, sin_sl[:])
                nc.vector.tensor_sub(dst[0:64], ta[:], tb[:])
                tc_ = work.tile([64, TQ], f32, tag="rope_a", bufs=2)
                td = work.tile([64, TQ], f32, tag="rope_b", bufs=2)
                nc.vector.tensor_mul(tc_[:], t0, sin_sl[:])
                nc.vector.tensor_mul(td[:], t1, cos_sl[:])
                nc.vector.tensor_add(dst[64:128], tc_[:], td[:])

            # ---- phase A1: all projections, one 512-token slot at a time ----
            # Slot 0 is special: the initial DMA burst (~11MB) is what paces
            # the PE, so all six projection chains run chunk-outer — each
            # arriving c-chunk feeds 6 matmuls (~1.6us) instead of one.
            xt_t, cos_sl, sin_sl = xt0
            ps_kv0 = pp.tile([128, 2, TQ], f32, tag="mm", name="ps_kv0")
            ps_qa = [
                pacc.tile([128, TQ], f32, tag="acc", name=f"ps0_q{m}")[:]
                for m in range(2)
            ] + [
                psums.tile([128, TQ], f32, tag="sums", name=f"ps0_q{m+2}")[:]
                for m in range(2)
            ]
            for c in range(NKC):
                nc.tensor.matmul(
                    ps_kv0[:, 0, :], wk_sb[:, c, :], xt_t[:, c, :],
                    start=(c == 0), stop=(c == NKC - 1),
                )
                nc.tensor.matmul(
                    ps_kv0[:, 1, :], wv_sb[:, c, :], xt_t[:, c, :],
                    start=(c == 0), stop=(c == NKC - 1),
                )
                for hh in range(NH_LOC):
                    nc.tensor.matmul(
                        ps_qa[hh], wq_sb[:, c, hh * HD : (hh + 1) * HD],
                        xt_t[:, c, :],
                        start=(c == 0), stop=(c == NKC - 1),
                    )
            rope(kres[:, 0:TQ], ps_kv0[:, 0, :], cos_sl, sin_sl)
            vt0_sb = work.tile([128, TQ], bf16, tag="vt", name="vt0")
            nc.any.tensor_copy(out=vt0_sb[:], in_=ps_kv0[:, 1, :])
            for hh in range(NH_LOC):
                qw = work.tile([128, TQ], bf16, tag="qw", bufs=2,
                               name=f"qw0_{hh}")
                rope(qw[:], ps_qa[hh], cos_sl, sin_sl)
                nc.scalar.dma_start(qres[hh, 0], qw[:])
            ps_tr0 = pp.tile([128, TQ], bf16, tag="mm", name="ps_tr0")
            for t in range(TQ // 128):
                nc.tensor.transpose(
                    ps_tr0[:, t * 128 : (t + 1) * 128],
                    vt0_sb[:, t * 128 : (t + 1) * 128],
                    ident_sb[:],
                )
            nc.any.tensor_copy(out=vres[:, 0:4, :], in_=ps_tr0[:])

            for j in range(1, NSLOT):
                xt_t, cos_sl, sin_sl = load_xt(j)

                # K^T
                ps_k = pp.tile([128, TQ], f32, tag="mm")
                for c in range(NKC):
                    nc.tensor.matmul(
                        ps_k[:], wk_sb[:, c, :], xt_t[:, c, :],
                        start=(c == 0), stop=(c == NKC - 1),
                    )
                rope(kres[:, j * TQ : (j + 1) * TQ], ps_k, cos_sl, sin_sl)

                # V in natural [tok, hd] layout directly: xt chunks stationary
                # against wv moving (N=128 chains pace at ~67ns) — replaces
                # the V^T projection + PE transpose + two evacuation copies
                ps_v = pp.tile([128, TQ], f32, tag="mm")
                for t in range(TQ // 128):
                    for c in range(NKC):
                        nc.tensor.matmul(
                            ps_v[:, t * 128 : (t + 1) * 128],
                            xt_t[:, c, t * 128 : (t + 1) * 128],
                            wv_sb[:, c, :],
                            start=(c == 0), stop=(c == NKC - 1),
                            skip_group_check=True,
                        )
                nc.any.tensor_copy(
                    out=vres[:, j * 4 : j * 4 + 4, :], in_=ps_v[:]
                )

                ps_q = pp.tile([128, TQ], f32, tag="mm")
                for c in range(NKC):
                    nc.tensor.matmul(
                        ps_q[:], wq_sb[:, c, 0:HD], xt_t[:, c, :],
                        start=(c == 0), stop=(c == NKC - 1),
                    )
                qw = work.tile([128, TQ], bf16, tag="qw", bufs=2, name=f"qw{j}_0")
                rope(qw[:], ps_q, cos_sl, sin_sl)
                nc.scalar.dma_start(qres[0, j], qw[:])

                for h in range(1, NH_LOC):
                    ps_q = pp.tile([128, TQ], f32, tag="mm", name=f"ps_q{j}_{h}")
                    for c in range(NKC):
                        nc.tensor.matmul(
                            ps_q[:], wq_sb[:, c, h * HD : (h + 1) * HD],
                            xt_t[:, c, :],
                            start=(c == 0), stop=(c == NKC - 1),
                        )
                    qw = work.tile([128, TQ], bf16, tag="qw", bufs=2,
                                   name=f"qw{j}_{h}")
                    rope(qw[:], ps_q, cos_sl, sin_sl)
                    nc.scalar.dma_start(qres[h, j], qw[:])

            # ---- phase A2: attention, head-outer; fire A2A per head ----
            xtctx.close()
            aoctx = ExitStack()
            aop = aoctx.enter_context(tc.tile_pool(name="aop", bufs=1))
            wop = aoctx.enter_context(tc.tile_pool(name="wop", bufs=2))

            def load_wo_half(nb, half):
                # rows for head-groups {2*half, 2*half+1}, all 8 source cores:
                # one strided DMA (per-partition 8 blocks of 2KB)
                wo_t = wop.tile([128, N_CORES, 2, TQ], bf16, tag="wot",
                                name=f"wo{nb}_{half}")
                nc.sync.dma_start(
                    wo_t[:], wo[nb, :, :, 2 * half : 2 * half + 2, :]
                )
                return wo_t

            ao_sb = {}

            def emit_collective(h):
                # trigger only; the SBUF copy of a2a_out happens in phase C
                # (so no engine queue ever blocks on an in-flight collective)
                nc.gpsimd.collective_compute(
                    "AllToAll",
                    mybir.AluOpType.bypass,
                    replica_groups=[list(range(N_CORES))],
                    ins=[a2a_in[h].opt()],
                    outs=[a2a_out[h].opt()],
                )

            def copy_ao(h, eng=None):
                ao_t = aop.tile([128, N_CORES, TQ], bf16, name=f"ao_sb{h}")
                for s_ in range(N_CORES):
                    e = eng or (nc.scalar if s_ % 2 == 0 else nc.gpsimd)
                    e.dma_start(ao_t[:, s_, :], a2a_out[h][s_])
                ao_sb[h] = ao_t

            def make_scores_pair(h, j, q_sl, kb0, qb):
                def scores_pair(pi):
                    """one exp for two key blocks: [128, 2, TQ]"""
                    ps_s = pp.tile([128, 2, TQ], f32, tag="mm",
                                   name=f"ps_s{h}_{j}_{pi}")
                    for i in range(2):
                        kk = kb0 + 2 * pi + i
                        nc.tensor.matmul(
                            ps_s[:, i, :],
                            kres[:, kk * TKB : (kk + 1) * TKB],
                            q_sl, start=True, stop=True,
                        )
                    p_sb = work.tile([128, 2, TQ], bf16, tag="p",
                                     name=f"p_sb{h}_{j}_{pi}")
                    if mode == "masked":
                        smask = work.tile([128, 2, TQ], f32, tag="smask")
                        for i in range(2):
                            kb = 2 * pi + i
                            nc.sync.dma_start(
                                smask[:, i, :],
                                maskT[kb * TKB : (kb + 1) * TKB,
                                      qb * TQ : (qb + 1) * TQ],
                            )
                        tmp_s = work.tile([128, 2, TQ], f32, tag="tmps")
                        nc.vector.tensor_scalar_mul(tmp_s[:], ps_s[:], SCALE)
                        nc.vector.tensor_add(tmp_s[:], tmp_s[:], smask[:])
                        nc.scalar.activation(
                            p_sb[:], tmp_s[:],
                            mybir.ActivationFunctionType.Exp,
                        )
                    else:
                        nc.scalar.activation(
                            p_sb[:], ps_s[:],
                            mybir.ActivationFunctionType.Exp,
                            scale=SCALE,
                        )
                    return p_sb
                return scores_pair

            qpre = {}

            def load_q(h, b, qb):
                # issued one slot ahead by the driver so a head boundary
                # never stalls on the 128KB q fetch
                key = (h, b, qb)
                if key not in qpre:
                    j = b * NQB + qb
                    q_t = work.tile([128, TQ], bf16, tag="q", bufs=3,
                                    name=f"q{h}_{j}")
                    nc.sync.dma_start(q_t[:], qres[h, j])
                    qpre[key] = q_t
                return qpre[key]

            def emit_slot_causal(h, b, qb):
                """Emit HEAD (q load, diag wave 1, first two pairs); return
                (mid, tail) closures. The driver emits H(n), T(n-1), M(n) so
                the latency-bound diagonal tail of each slot overlaps the
                next slot's independent scores work."""
                j = b * NQB + qb
                kb0 = b * (S // TKB)
                dkb = kb0 + 4 * qb
                npairs = 2 * qb
                q_t = load_q(h, b, qb)
                del qpre[(h, b, qb)]
                q_sl = q_t[:]
                scores_pair = make_scores_pair(h, j, q_sl, kb0, qb)
                p_acc = work.tile([128, TQ], bf16, tag="pacc", bufs=2,
                                  name=f"pacc{h}_{j}")
                p_d1 = work.tile([128, 2, TQ], bf16, tag="pd", bufs=3,
                                 name=f"pd1_{h}_{j}")
                p_d2 = work.tile([128, 2, TQ], bf16, tag="pd", bufs=3,
                                 name=f"pd2_{h}_{j}")
                d_ps = []
                state = {}

                def diag_scores(kb):
                    W = TQ - 128 * kb
                    dps = psums.tile([128, TQ], f32, tag="sums",
                                     name=f"dps{h}_{j}_{kb}")
                    nc.tensor.matmul(
                        dps[:, 0:W],
                        kres[:, (dkb + kb) * TKB : (dkb + kb + 1) * TKB],
                        q_sl[:, 128 * kb : TQ],
                        start=True, stop=False,
                        skip_group_check=True,
                    )
                    nc.tensor.matmul(
                        dps[:, 0:128], masktri_sb[:], ident_sb[:],
                        start=False, stop=True,
                        skip_group_check=True,
                    )
                    return dps

                def diag_exp(kb, dst):
                    W = TQ - 128 * kb
                    nc.scalar.activation(
                        dst[:, 0:W], d_ps[kb][:, 0:W],
                        mybir.ActivationFunctionType.Exp,
                        scale=SCALE,
                    )

                # HEAD: diag wave 1 first so its exps land early on the ACT
                # queue, then the first two off-diag pairs
                plist = []
                d_ps.append(diag_scores(0))
                d_ps.append(diag_scores(1))
                diag_exp(0, p_d1[:, 0, :])
                if npairs > 0:
                    plist.append(scores_pair(0))
                diag_exp(1, p_d1[:, 1, :])
                if npairs >= 2:
                    plist.append(scores_pair(1))

                def mid():
                    ps_o = pacc.tile([128, TQ], f32, tag="acc",
                                     name=f"ps_o{h}_{j}")
                    state["ps_o"] = ps_o

                    def wave2():
                        # kb2 (W=256) and kb3 (W=128) packed into one PSUM
                        # bank -> a single exp covers both (saves the ~300ns
                        # fixed ACT overhead of a 4th activation per slot)
                        t2 = psums.tile([128, TQ], f32, tag="sums",
                                        name=f"dps{h}_{j}_w2")
                        nc.tensor.matmul(
                            t2[:, 0:256],
                            kres[:, (dkb + 2) * TKB : (dkb + 3) * TKB],
                            q_sl[:, 256:TQ],
                            start=True, stop=False,
                            skip_group_check=True,
                        )
                        nc.tensor.matmul(
                            t2[:, 0:128], masktri_sb[:], ident_sb[:],
                            start=False, stop=True,
                            skip_group_check=True,
                        )
                        nc.tensor.matmul(
                            t2[:, 256:384],
                            kres[:, (dkb + 3) * TKB : (dkb + 4) * TKB],
                            q_sl[:, 384:TQ],
                            start=True, stop=False,
                            skip_group_check=True,
                        )
                        nc.tensor.matmul(
                            t2[:, 256:384], masktri_sb[:], ident_sb[:],
                            start=False, stop=True,
                            skip_group_check=True,
                        )
                        nc.scalar.activation(
                            p_d2[:, 0, 0:384], t2[:, 0:384],
                            mybir.ActivationFunctionType.Exp,
                            scale=SCALE,
                        )

                    w2_done = False
                    for pi in range(npairs):
                        if pi + 2 < npairs:
                            plist.append(scores_pair(pi + 2))
                        if pi == 2 and not w2_done:
                            wave2()
                            w2_done = True
                        p_cur = plist[pi]
                        for i in range(2):
                            nc.tensor.matmul(
                                ps_o[:], vres[:, kb0 + 2 * pi + i, :],
                                p_cur[:, i, :],
                                start=(pi == 0 and i == 0), stop=False,
                                skip_group_check=True,
                            )
                        if pi == 0:
                            nc.vector.tensor_add(
                                p_acc[:], p_cur[:, 0, :], p_cur[:, 1, :]
                            )
                        else:
                            for i in range(2):
                                nc.vector.tensor_add(
                                    p_acc[:], p_acc[:], p_cur[:, i, :]
                                )
                    if not w2_done:
                        wave2()

                def tail():
                    ps_o = state["ps_o"]
                    # denominator accumulation over the diag tiles
                    if npairs == 0:
                        nc.vector.tensor_copy(
                            out=p_acc[:], in_=p_d1[:, 0, :]
                        )
                    else:
                        nc.vector.tensor_add(
                            p_acc[:], p_acc[:], p_d1[:, 0, :]
                        )
                    nc.vector.tensor_add(
                        p_acc[:, 128:TQ], p_acc[:, 128:TQ],
                        p_d1[:, 1, 0:384],
                    )
                    nc.vector.tensor_add(
                        p_acc[:, 256:TQ], p_acc[:, 256:TQ],
                        p_d2[:, 0, 0:256],
                    )
                    nc.vector.tensor_add(
                        p_acc[:, 384:TQ], p_acc[:, 384:TQ],
                        p_d2[:, 0, 256:384],
                    )

                    # diag PV. The PE accumulation group is stateful:
                    # interleaved sub-range groups in one bank corrupt data,
                    # so qb==0 (no prior full-width write) runs qc-outer with
                    # each 128-wide chain opened and closed contiguously;
                    # qb>=1 accumulates into the group opened by the off-diag
                    # full-width write.
                    def dsrc(kb):
                        if kb < 2:
                            return p_d1[:, kb, :]
                        if kb == 2:
                            return p_d2[:, 0, :]
                        return p_d2[:, 0, 256:384]

                    if npairs == 0:
                        for qc in range(4):
                            for kb in range(qc + 1):
                                off = 128 * (qc - kb)
                                nc.tensor.matmul(
                                    ps_o[:, 128 * qc : 128 * (qc + 1)],
                                    vres[:, dkb + kb, :],
                                    dsrc(kb)[:, off : off + 128],
                                    start=(kb == 0), stop=(kb == qc),
                                    skip_group_check=True,
                                )
                    else:
                        for kb in range(4):
                            for qc in range(kb, 4):
                                off = 128 * (qc - kb)
                                nc.tensor.matmul(
                                    ps_o[:, 128 * qc : 128 * (qc + 1)],
                                    vres[:, dkb + kb, :],
                                    dsrc(kb)[:, off : off + 128],
                                    start=False, stop=(kb == qc),
                                    skip_group_check=True,
                                )

                    ps_sum_t = psums.tile([128, TQ], f32, tag="sums",
                                          name=f"ps_sum{h}_{j}")
                    ps_sum = ps_sum_t[:]
                    nc.tensor.matmul(
                        ps_sum, ones_sb[:], p_acc[:],
                        start=True, stop=True,
                    )
                    recip = work.tile([128, TQ], f32, tag="recip", bufs=2,
                                      name=f"recip{h}_{j}")
                    nc.vector.reciprocal_approx_fast(recip[:], ps_sum)
                    o_sb = work.tile([128, TQ], bf16, tag="o", bufs=2,
                                     name=f"o{h}_{j}")
                    nc.vector.tensor_mul(o_sb[:], ps_o[:], recip[:])
                    nc.sync.dma_start(a2a_in[h][j], o_sb[:])

                return mid, tail

            wo_pre = {}
            if mode == "causal":
                # Slot order: descending qb, batches interleaved. Fat slots
                # (many independent score pairs) lead each head, so a head
                # boundary always overlaps the previous thin tail with fresh
                # PE work; the final tail (qb=0) is tiny and phase C covers it.
                slot_order = [(b, qb) for qb in reversed(range(NQB))
                              for b in range(B)]
                prev_tail = None
                for h in range(NH_LOC):
                    if h == 2:
                        wo_pre[(0, 0)] = load_wo_half(0, 0)
                    if h == 3:
                        wo_pre[(1, 0)] = load_wo_half(1, 0)
                    for si, (b, qb) in enumerate(slot_order):
                        mid, tailf = emit_slot_causal(h, b, qb)
                        # prefetch the next slot's q (crosses head boundary)
                        if si + 1 < len(slot_order):
                            load_q(h, *slot_order[si + 1])
                        elif h + 1 < NH_LOC:
                            load_q(h + 1, *slot_order[0])
                        if prev_tail is not None:
                            prev_tail()
                            if si == 0 and h > 0:
                                emit_collective(h - 1)
                        mid()
                        prev_tail = tailf
                prev_tail()
                emit_collective(NH_LOC - 1)
            else:
                for h in range(NH_LOC):
                    if h == NH_LOC - 1:
                        wo_pre[(0, 0)] = load_wo_half(0, 0)
                        wo_pre[(1, 0)] = load_wo_half(1, 0)
                    for b in range(B):
                        for qb in range(NQB):
                            j = b * NQB + qb
                            q_t = work.tile([128, TQ], bf16, tag="q", bufs=2,
                                            name=f"q{h}_{j}")
                            nc.sync.dma_start(q_t[:], qres[h, j])
                            q_sl = q_t[:]
                            nkb = 4 * NQB
                            kb0 = b * (S // TKB)
                            scores_pair = make_scores_pair(h, j, q_sl, kb0, qb)

                            ps_o = pacc.tile([128, TQ], f32, tag="acc",
                                             name=f"ps_o{h}_{j}")
                            p_acc = work.tile([128, TQ], bf16, tag="pacc",
                                              bufs=2, name=f"pacc{h}_{j}")
                            # Off-diagonal: 2*qb full pairs with 2-deep
                            # prefetch (the exp chain runs ~95% of PE rate, so
                            # depth 2 is needed to hide its latency).
                            # Diagonal: 4 key blocks truncated to the causal
                            # triangle (widths 512/384/256/128); the in-block
                            # triangle gets -1e9 added via an accumulating
                            # matmul (masktri @ I) before the exp, so exp
                            # yields exact zeros and no DVE masking is needed.
                            npairs = 2 * qb
                            dkb = kb0 + 4 * qb
                            d_ps = []
                            p_d1 = work.tile([128, 2, TQ], bf16, tag="pd",
                                             bufs=2, name=f"pd1_{h}_{j}")
                            p_d2 = work.tile([128, 2, TQ], bf16, tag="pd",
                                             bufs=2, name=f"pd2_{h}_{j}")

                            def diag_scores(kb, jj=j, q_sl=q_sl, dkb=dkb):
                                W = TQ - 128 * kb
                                dps = psums.tile([128, TQ], f32, tag="sums",
                                                 name=f"dps{jj}_{kb}")
                                nc.tensor.matmul(
                                    dps[:, 0:W],
                                    kres[:, (dkb + kb) * TKB :
                                         (dkb + kb + 1) * TKB],
                                    q_sl[:, 128 * kb : TQ],
                                    start=True, stop=False,
                                    skip_group_check=True,
                                )
                                nc.tensor.matmul(
                                    dps[:, 0:128], masktri_sb[:], ident_sb[:],
                                    start=False, stop=True,
                                    skip_group_check=True,
                                )
                                return dps

                            def diag_exp(kb, dst):
                                W = TQ - 128 * kb
                                nc.scalar.activation(
                                    dst[:, 0:W], d_ps[kb][:, 0:W],
                                    mybir.ActivationFunctionType.Exp,
                                    scale=SCALE,
                                )

                            def wave2():
                                d_ps.append(diag_scores(2))
                                d_ps.append(diag_scores(3))
                                diag_exp(2, p_d2[:, 0, :])
                                diag_exp(3, p_d2[:, 1, :])

                            plist = []
                            if npairs > 0:
                                plist.append(scores_pair(0))
                            d_ps.append(diag_scores(0))
                            d_ps.append(diag_scores(1))
                            diag_exp(0, p_d1[:, 0, :])
                            if npairs >= 2:
                                plist.append(scores_pair(1))
                            diag_exp(1, p_d1[:, 1, :])

                            w2_done = False
                            for pi in range(npairs):
                                if pi + 2 < npairs:
                                    plist.append(scores_pair(pi + 2))
                                if pi == 2 and not w2_done:
                                    wave2()
                                    w2_done = True
                                p_cur = plist[pi]
                                for i in range(2):
                                    nc.tensor.matmul(
                                        ps_o[:], vres[:, kb0 + 2 * pi + i, :],
                                        p_cur[:, i, :],
                                        start=(pi == 0 and i == 0), stop=False,
                                        skip_group_check=True,
                                    )
                                if pi == 0:
                                    nc.vector.tensor_add(
                                        p_acc[:], p_cur[:, 0, :], p_cur[:, 1, :]
                                    )
                                else:
                                    for i in range(2):
                                        nc.vector.tensor_add(
                                            p_acc[:], p_acc[:], p_cur[:, i, :]
                                        )
                            if not w2_done:
                                wave2()

                            # denominator accumulation over the diag tiles
                            if npairs == 0:
                                nc.vector.tensor_copy(
                                    out=p_acc[:], in_=p_d1[:, 0, :]
                                )
                            else:
                                nc.vector.tensor_add(
                                    p_acc[:], p_acc[:], p_d1[:, 0, :]
                                )
                            nc.vector.tensor_add(
                                p_acc[:, 128:TQ], p_acc[:, 128:TQ],
                                p_d1[:, 1, 0:384],
                            )
                            nc.vector.tensor_add(
                                p_acc[:, 256:TQ], p_acc[:, 256:TQ],
                                p_d2[:, 0, 0:256],
                            )
                            nc.vector.tensor_add(
                                p_acc[:, 384:TQ], p_acc[:, 384:TQ],
                                p_d2[:, 1, 0:128],
                            )

                            # diag PV. The PE accumulation group is stateful:
                            # interleaved sub-range groups in one bank corrupt
                            # data, so qb==0 (no prior full-width write) runs
                            # qc-outer with each 128-wide chain opened and
                            # closed contiguously; qb>=1 accumulates into the
                            # group opened by the off-diag full-width write.
                            def dsrc(kb):
                                return p_d1[:, kb, :] if kb < 2 else \
                                    p_d2[:, kb - 2, :]

                            if npairs == 0:
                                for qc in range(4):
                                    for kb in range(qc + 1):
                                        off = 128 * (qc - kb)
                                        nc.tensor.matmul(
                                            ps_o[:, 128 * qc : 128 * (qc + 1)],
                                            vres[:, dkb + kb, :],
                                            dsrc(kb)[:, off : off + 128],
                                            start=(kb == 0), stop=(kb == qc),
                                            skip_group_check=True,
                                        )
                            else:
                                for kb in range(4):
                                    for qc in range(kb, 4):
                                        off = 128 * (qc - kb)
                                        nc.tensor.matmul(
                                            ps_o[:, 128 * qc : 128 * (qc + 1)],
                                            vres[:, dkb + kb, :],
                                            dsrc(kb)[:, off : off + 128],
                                            start=False, stop=(kb == qc),
                                            skip_group_check=True,
                                        )

                            ps_sum_t = pp.tile([128, 2, TQ], f32, tag="mm",
                                               name=f"ps_sum{j}")
                            ps_sum = ps_sum_t[:, 0, :]
                        else:
                            # dense/masked: full blocks, 1-deep prefetch
                            p_sb01 = scores_pair(0)
                            npairs = nkb // 2
                            p_cur = scores_pair(1)
                            for i in range(2):
                                nc.tensor.matmul(
                                    ps_o[:], vres[:, kb0 + i, :],
                                    p_sb01[:, i, :],
                                    start=(i == 0), stop=False,
                                )
                            nc.vector.tensor_add(
                                p_acc[:], p_sb01[:, 0, :], p_sb01[:, 1, :]
                            )
                            for pi in range(1, npairs):
                                p_next = (scores_pair(pi + 1)
                                          if pi + 1 < npairs else None)
                                for i in range(2):
                                    kb = 2 * pi + i
                                    kk = kb0 + kb
                                    nc.tensor.matmul(
                                        ps_o[:], vres[:, kk, :], p_cur[:, i, :],
                                        start=False, stop=(kb == nkb - 1),
                                    )
                                    nc.vector.tensor_add(
                                        p_acc[:], p_acc[:], p_cur[:, i, :]
                                    )
                                p_cur = p_next
                            ps_sum_t = psums.tile([128, TQ], f32, tag="sums",
                                                  name=f"ps_sum{j}")
                            ps_sum = ps_sum_t[:]

                        nc.tensor.matmul(
                            ps_sum, ones_sb[:], p_acc[:],
                            start=True, stop=True,
                        )
                        recip = work.tile([128, TQ], f32, tag="recip", bufs=2)
                        nc.vector.reciprocal_approx_fast(recip[:], ps_sum)
                        o_sb = work.tile([128, TQ], bf16, tag="o", bufs=2)
                        nc.vector.tensor_mul(o_sb[:], ps_o[:], recip[:])
                        nc.sync.dma_start(a2a_in[h][j], o_sb[:])

                emit_collective(h)

            # ---- phase C: out[my 512 tokens] = AO @ wo (full wo) ----
            # Two passes so no matmul ever waits on a late AllToAll:
            #   pass A: hg0+hg1 for every nb -> bf16 partial in SBUF
            #           (needs only collectives 0/1; ~130us of runway)
            #   pass B: hg2+hg3 -> PSUM; the flush adds the partial back in
            #           (collectives 2/3 long landed)
            with (
                tc.tile_pool(name="outp", bufs=3) as outp,
                tc.tile_pool(name="partp", bufs=1) as partp,
            ):
                partial = partp.tile([128, D // TQ, 4, TQ], bf16)

                def alloc_ps_out(nb):
                    if nb % 2 == 0:
                        pair_a = pp.tile([128, 2, TQ], f32, tag="mm",
                                         name=f"ps_outa{nb}")
                        pair_b = pp.tile([128, 2, TQ], f32, tag="mm",
                                         name=f"ps_outb{nb}")
                        return [pair_a[:, 0, :], pair_a[:, 1, :],
                                pair_b[:, 0, :], pair_b[:, 1, :]]
                    return [
                        pacc.tile([128, TQ], f32, tag="acc",
                                  name=f"ps_oa{nb}_{m}")[:]
                        for m in range(2)
                    ] + [
                        psums.tile([128, TQ], f32, tag="sums",
                                   name=f"ps_ob{nb}_{m}")[:]
                        for m in range(2)
                    ]

                def mm_half(ps_out, wo_t, half, first, close):
                    for hgi in range(2):
                        hg = 2 * half + hgi
                        for s_ in range(N_CORES):
                            last = close and hgi == 1 and s_ == N_CORES - 1
                            for m in range(4):
                                nc.tensor.matmul(
                                    ps_out[m],
                                    ao_sb[hg][:, s_, m * 128 : (m + 1) * 128],
                                    wo_t[:, s_, hgi, :],
                                    start=first, stop=last,
                                )
                            first = False

                def flush_partial(nb, ps_out):
                    # bf16 partial: one rounding of the hg0+hg1 half-sum
                    # (~0.3% of |out|, well within the error budget)
                    for m in range(4):
                        if m % 2 == 0:
                            nc.vector.tensor_copy(
                                out=partial[:, nb, m, :], in_=ps_out[m]
                            )
                        else:
                            nc.scalar.copy(partial[:, nb, m, :], ps_out[m])

                def flush_out(nb, ps_out):
                    # osb = pass-B PSUM + pass-A partial (replaces re-opening
                    # the accumulation with identity matmuls: saves PE work)
                    for m in range(4):
                        osb = outp.tile([128, TQ], f32, tag="osb",
                                        name=f"osb{nb}_{m}")
                        nc.vector.tensor_add(
                            osb[:], ps_out[m], partial[:, nb, m, :]
                        )
                        deng = nc.sync if m % 2 == 0 else nc.scalar
                        deng.dma_start(
                            out[m * 128 : (m + 1) * 128, nb * TQ : (nb + 1) * TQ],
                            osb[:],
                        )

                # pass A (head-groups 0/1); gpsimd queue only (it is idle
                # here — the Tile scheduler may hoist these collective-gated
                # DMAs, which must never block a queue with real work)
                copy_ao(0, nc.gpsimd)
                copy_ao(1, nc.gpsimd)
                for nb in range(D // TQ):
                    if (nb, 0) in wo_pre:
                        wo_t = wo_pre.pop((nb, 0))
                    else:
                        wo_t = load_wo_half(nb, 0)
                    ps_out = alloc_ps_out(nb)
                    mm_half(ps_out, wo_t, 0, True, True)
                    flush_partial(nb, ps_out)
                # pass B (head-groups 2/3)
                copy_ao(2, nc.gpsimd)
                copy_ao(3, nc.gpsimd)
                for nb in range(D // TQ):
                    wo_t = load_wo_half(nb, 1)
                    ps_out = alloc_ps_out(nb)
                    mm_half(ps_out, wo_t, 1, True, True)
                    flush_out(nb, ps_out)
            aoctx.close()
            psctx.close()
            actx.close()

    nc.finalize()
    return nc


def _detect_mode(mask: np.ndarray) -> str:
    if not np.any(mask):
        return "dense"
    tril_ok = not np.any(mask[np.tril_indices(S)])
    iu = np.triu_indices(S, 1)
    triu_ok = np.all(mask[iu] <= -1e8)
    if tril_ok and triu_ok:
        return "causal"
    return "masked"


def kernel(x, wq, wk, wv, wo, cache_k, cache_v, freqs_cos, freqs_sin, mask,
           start_pos):
    from ml_dtypes import bfloat16

    from concourse.bass_utils import run_bass_kernel_spmd

    assert int(start_pos) == 0, "kernel hardcodes start_pos == 0"
    x = np.asarray(x, dtype=np.float32)
    wq = np.asarray(wq, dtype=np.float32)
    wk = np.asarray(wk, dtype=np.float32)
    wv = np.asarray(wv, dtype=np.float32)
    wo = np.asarray(wo, dtype=np.float32)
    freqs_cos = np.asarray(freqs_cos, dtype=np.float32)
    freqs_sin = np.asarray(freqs_sin, dtype=np.float32)
    mask = np.asarray(mask, dtype=np.float32)

    mode = _detect_mode(mask)
    if mode not in _NC_CACHE:
        _NC_CACHE[mode] = _build_nc(mode)
    nc = _NC_CACHE[mode]

    # X^T slot-tiled [8, 128, 32, 512]: [j, p, c, t] = x_flat[512j+t, 128c+p]
    x_flat = x.reshape(TOK, D)
    xT = np.ascontiguousarray(
        x_flat.T.reshape(NKC, 128, NSLOT, TQ).transpose(2, 1, 0, 3)
    ).astype(bfloat16)

    # de-interleave RoPE pairs within each head: [0,2,...,126,1,3,...,127]
    perm = np.concatenate([np.arange(0, HD, 2), np.arange(1, HD, 2)])

    # cos/sin transposed, tiled over batches: [64, 4096]
    cosT = np.ascontiguousarray(
        np.concatenate([freqs_cos.T] * B, axis=1), dtype=np.float32
    )
    sinT = np.ascontiguousarray(
        np.concatenate([freqs_sin.T] * B, axis=1), dtype=np.float32
    )

    # wo nb-tiled [8, 128, 8, 4, 512]: [nb, p, s_, hg, n] = wo[128(4s+g)+p, 512nb+n]
    wo_bf = np.ascontiguousarray(
        wo.reshape(NH, 128, D // TQ, TQ).transpose(2, 1, 0, 3)
    ).astype(bfloat16).reshape(D // TQ, 128, N_CORES, NH_LOC, TQ)

    def to_chunked(w):  # [4096, F] -> [128, 32, F]
        return np.ascontiguousarray(
            w.reshape(NKC, 128, w.shape[1]).transpose(1, 0, 2)
        ).astype(bfloat16)

    if mode == "causal":
        # masktri[c, p] = -1e9 if p > c (strict upper triangle)
        masktri = np.where(
            np.arange(128)[None, :] > np.arange(128)[:, None], -1e9, 0.0
        ).astype(bfloat16)

    in_maps = []
    for r in range(N_CORES):
        q_cols = np.concatenate(
            [(4 * r + h) * HD + perm for h in range(NH_LOC)]
        )
        m = {
            "xT": xT,
            "wq": to_chunked(wq[:, q_cols]),
            "wk": to_chunked(wk[:, r * HD + perm]),
            "wv": to_chunked(wv[:, r * HD : (r + 1) * HD]),
            "wo": wo_bf,
            "cosT": cosT,
            "sinT": sinT,
        }
        if mode == "causal":
            m["masktri"] = masktri
        if mode == "masked":
            m["maskT"] = np.ascontiguousarray(mask.T)
        in_maps.append(m)

    kwargs = {}
    if PROFILE and TRACE_DIR is not None:
        kwargs["tmpdir"] = TRACE_DIR
    res = run_bass_kernel_spmd(
        nc, in_maps, list(range(N_CORES)), trace=PROFILE, **kwargs
    )
    global LAST_EXEC_NS, LAST_TRACE_DIR
    LAST_EXEC_NS = res.exec_time_ns
    if PROFILE and res.profile_json is not None:
        LAST_TRACE_DIR = res.profile_json

    out_full = np.empty((TOK, D), dtype=np.float32)
    for r in range(N_CORES):
        out_full[r * TQ : (r + 1) * TQ] = res.results[r]["out"]
    return out_full.reshape(B, S, D)

